# revision 43
# baseline (speedup 1.0000x reference)
"""DualBranchCFCA Trainium2 kernel (v3 — engine-separated pipeline).

Math (per batch b):
    att_t = sigmoid(relu(mean_hw(x_t) @ w1_t + b1_t) @ w2_t + b2_t)      [ct]
    att_c = sigmoid(relu(mean_hw(x_c) @ w1_c + b1_c) @ w2_c + b2_c)      [cc]
    mask  = top_k(att_t, K) one-hot mask in {0,1}                        [ct]
    W     = softmax(cross_att, axis=-1)                                  [ct, cc]
    out_t = att_t * x_t + mask  * (W @ x_c)
    out_c = att_c * x_c + att_c * (W @ x_t)

Strategy: data-parallel over batch across 8 cores (2 batches/core), params
replicated.  Per core, strict engine separation so nothing blocks the input
stream:
  - sync engine: issues ONLY the input half-chunk (1 MB) DMAs -> landing ring.
  - ACT engine:  fp32->bf16 casts (with exact fp32 spatial sums via the
    activation accumulator) + the tiny SE MLP nonlinearities + softmax exps.
  - DVE: softmax scale + wt_full/psum-row evacuations + mask compares + the
    fused drain scalar_tensor_tensor (x*att + psum -> bf16 asm tiles),
    1024 cols (2 PSUM banks) per op.
  - gpsimd: partition broadcasts + output DMA issues (SWDGE queue).
  - PE: SE/rank/transpose smalls + the main bf16 GEMMs (k-outer weight reuse,
    fp32 PSUM accumulation over the 4 channel chunks).
Both batches' bf16 chunks stay resident so batch 1 loads stream during
batch 0 GEMMs.  Selection math (top-k over SE logits) is exact fp32.
"""

import os
from contextlib import ExitStack

import numpy as np

import concourse.bacc as bacc
import concourse.bass as bass
import concourse.mybir as mybir
import concourse.tile as tile
from concourse import masks
from concourse.bass_utils import run_bass_kernel_spmd

F32 = mybir.dt.float32
BF16 = mybir.dt.bfloat16
FP16 = mybir.dt.float16
AF = mybir.ActivationFunctionType
ALU = mybir.AluOpType
AX = mybir.AxisListType

N_CORES = 8
B_FULL = 16
B = B_FULL // N_CORES  # batches per core
C = 512                # channels (both branches)
HW = 64 * 64           # flattened spatial
RED = 16
H = C // RED           # SE hidden dim = 32
K_TOP = int(C * 0.3)   # 153
P = 128                # partitions
NCH = C // P           # 4 channel chunks
NHALF = 2              # landing halves per chunk
HALF = HW // NHALF     # 2048
NSP = HW // 512        # 8 spatial tiles of 512

_CACHE = {}
LAST_RESULTS = None  # BassKernelResults of the most recent run (for profiling)


def _se_branch(nc, pools, w1, b1, w2, b2, sums, pfx):
    """SE MLP from per-chunk spatial sums -> (z chunks, att chunks), each [128,1].

    z = sums/HW @ w1 -> relu(+b1) -> @ w2 + b2 (pre-sigmoid logits, exact fp32)
    att = sigmoid(z)
    """
    small, psmall = pools["small"], pools["psmall"]
    hz = psmall.tile([H, 1], F32, tag="ps")
    for i in range(NCH):
        nc.tensor.matmul(hz[:], w1[:, i, :], sums[i][:],
                         start=(i == 0), stop=(i == NCH - 1))
    h_sb = small.tile([H, 1], F32, tag=f"{pfx}h")
    nc.scalar.activation(h_sb[:], hz[:], AF.Relu, bias=b1[:], scale=1.0 / HW)

    z_chunks, att_chunks = [], []
    for j in range(NCH):
        zp = psmall.tile([P, 1], F32, tag="ps")
        nc.tensor.matmul(zp[:], w2[:, j * P:(j + 1) * P], h_sb[:])
        z = small.tile([P, 1], F32, tag=f"{pfx}z{j}")
        nc.scalar.activation(z[:], zp[:], AF.Identity, bias=b2[:, j:j + 1])
        att = small.tile([P, 1], F32, tag=f"{pfx}a{j}")
        nc.scalar.activation(att[:], z[:], AF.Sigmoid)
        z_chunks.append(z)
        att_chunks.append(att)
    return z_chunks, att_chunks


def _row_of(nc, pools, cols, ident, pfx, dtype):
    """Transpose NCH [128,1] column tiles into one [1, C] row tile."""
    small, psmall = pools["small"], pools["psmall"]
    row = small.tile([1, C], dtype, tag=f"{pfx}row")
    for j in range(NCH):
        tp = psmall.tile([1, P], F32, tag="ps")
        nc.tensor.transpose(tp[:], cols[j][:], ident[:])
        nc.vector.tensor_copy(row[:, j * P:(j + 1) * P], tp[:])
    return row


def build_program():
    nc = bacc.Bacc("TRN2", target_bir_lowering=False, debug=False)

    def din(name, shape):
        return nc.dram_tensor(name, shape, F32, kind="ExternalInput").ap()

    x_t = din("x_t", [B, C, 64, 64]).rearrange("b c h w -> b c (h w)")
    x_c = din("x_c", [B, C, 64, 64]).rearrange("b c h w -> b c (h w)")
    w1_t, b1_t = din("w1_t", [C, H]), din("b1_t", [H])
    w2_t, b2_t = din("w2_t", [H, C]), din("b2_t", [C])
    w1_c, b1_c = din("w1_c", [C, H]), din("b1_c", [H])
    w2_c, b2_c = din("w2_c", [H, C]), din("b2_c", [C])
    cross_att = din("cross_att", [C, C])

    out_t = nc.dram_tensor("out_t", [B, C, 64, 64], BF16,
                           kind="ExternalOutput").ap().rearrange("b c h w -> b c (h w)")
    out_c = nc.dram_tensor("out_c", [B, C, 64, 64], BF16,
                           kind="ExternalOutput").ap().rearrange("b c h w -> b c (h w)")

    with tile.TileContext(nc) as tc:
        with ExitStack() as ctx:
            _body(ctx, tc, x_t, x_c, w1_t, b1_t, w2_t, b2_t,
                  w1_c, b1_c, w2_c, b2_c, cross_att, out_t, out_c)
    nc.compile()
    return nc


def _body(ctx, tc, x_t, x_c, w1_t, b1_t, w2_t, b2_t,
          w1_c, b1_c, w2_c, b2_c, cross_att, out_t, out_c):
    nc = tc.nc
    const = ctx.enter_context(tc.tile_pool(name="const", bufs=1))
    small = ctx.enter_context(tc.tile_pool(name="small", bufs=2))
    med = ctx.enter_context(tc.tile_pool(name="med", bufs=2))
    wm_pool = ctx.enter_context(tc.tile_pool(name="wm", bufs=1))
    land_pool = ctx.enter_context(tc.tile_pool(name="land", bufs=3))
    xt_pool = ctx.enter_context(tc.tile_pool(name="xt", bufs=2 * NCH))
    xc_pool = ctx.enter_context(tc.tile_pool(name="xc", bufs=2 * NCH))
    asm_pool = ctx.enter_context(tc.tile_pool(name="asm", bufs=2))
    psmall = ctx.enter_context(tc.tile_pool(name="psmall", bufs=2, space="PSUM"))
    gpsum = ctx.enter_context(tc.tile_pool(name="gpsum", bufs=3, space="PSUM"))
    pools = {"small": small, "psmall": psmall}

    # ---- constants ----
    ident = const.tile([P, P], F32)
    masks.make_identity(nc, ident[:])
    ones_col = const.tile([P, 1], FP16)
    nc.vector.memset(ones_col[:], 1.0)
    # dummy broadcast at t=0: preloads the gpsimd Q7 partition_broadcast
    # program during the idle head so batch 0's mask-chain broadcast doesn't
    # pay the ~9us first-use program-load stall on the critical path
    warm_bc = const.tile([P, P], F32, tag="warmbc")
    nc.gpsimd.partition_broadcast(warm_bc[:], ident[0:1, :])

    # SE weights: w1 as [128, NCH, H] (lhsT chunks over contraction dim c),
    # w2 as [H, C] (lhsT over contraction dim h), biases as columns.
    def load_se(w1d, b1d, w2d, b2d, pfx):
        w1 = const.tile([P, NCH, H], F32, tag=f"{pfx}w1")
        nc.sync.dma_start(w1[:], w1d.rearrange("(k p) h -> p k h", p=P))
        b1 = const.tile([H, 1], F32, tag=f"{pfx}b1")
        nc.sync.dma_start(b1[:], b1d.unsqueeze(1))
        w2 = const.tile([H, C], F32, tag=f"{pfx}w2")
        nc.sync.dma_start(w2[:], w2d)
        b2 = const.tile([P, NCH], F32, tag=f"{pfx}b2")
        nc.sync.dma_start(b2[:], b2d.rearrange("(k p) -> p k", p=P))
        return w1, b1, w2, b2

    w1t, b1t, w2t, b2t = load_se(w1_t, b1_t, w2_t, b2_t, "t")
    w1c, b1c, w2c, b2c = load_se(w1_c, b1_c, w2_c, b2_c, "c")

    # ---- softmax(cross_att), transpose -> wt_full[c_part, j, t] (bf16) ----
    # No max-subtraction needed: |cross_att| <~ 5 so exp stays in fp32 range
    # and softmax is shift-invariant.  Transposes ride the big psum ring and
    # evacuate on DVE so the ACT engine stays free for the input casts.
    wt_full = const.tile([P, NCH, C], BF16, tag="wt")
    for i in range(NCH):
        ca = med.tile([P, C], F32, tag="zbc")
        nc.sync.dma_start(ca[:], cross_att[i * P:(i + 1) * P, :])
        sumexp = small.tile([P, 1], F32, tag="sumexp")
        nc.scalar.activation(ca[:], ca[:], AF.Exp, accum_out=sumexp[:])
        rec = small.tile([P, 1], F32, tag="rec")
        nc.vector.reciprocal(rec[:], sumexp[:])
        nc.vector.tensor_scalar_mul(ca[:], ca[:], rec[:])
        for j in range(NCH):
            tp = gpsum.tile([P, P], F32, tag="g")
            nc.tensor.transpose(tp[:], ca[:, j * P:(j + 1) * P], ident[:])
            nc.vector.tensor_copy(wt_full[:, j, i * P:(i + 1) * P], tp[:])

    def load_chunks(xdram, b, pool, pfx):
        """Stream half-chunks (1 MB) through the landing ring on the sync
        HWDGE queue; one fused ACT op per half casts to bf16 and emits the
        exact fp32 half-sum (ACT accumulates pre-cast)."""
        bf_chunks, half_sums = [], []
        for i in range(NCH):
            xb = pool.tile([P, HW], BF16, tag=f"{pfx}bf")
            halves = []
            for hh in range(NHALF):
                land = land_pool.tile([P, HALF], F32, tag="land")
                nc.sync.dma_start(
                    land[:], xdram[b, i * P:(i + 1) * P,
                                   hh * HALF:(hh + 1) * HALF])
                sh = small.tile([P, 1], F32, tag=f"{pfx}sh{i}{hh}")
                nc.scalar.activation(xb[:, hh * HALF:(hh + 1) * HALF], land[:],
                                     AF.Copy, accum_out=sh[:])
                halves.append(sh)
            bf_chunks.append(xb)
            half_sums.append(halves)
        return bf_chunks, half_sums

    def make_sums(half_sums, pfx):
        sums = []
        for i, halves in enumerate(half_sums):
            s = small.tile([P, 1], F32, tag=f"{pfx}s{i}")
            nc.vector.tensor_add(s[:], halves[0][:], halves[1][:])
            sums.append(s)
        return sums

    def gemm_job(b, wm, rhs, atts, xdir, odram, out_eng=None):
        # out[m,n] = atts[m]*xdir[m,n] + sum_k wm[k,m] @ rhs[k,n]
        out_eng = out_eng or nc.gpsimd
        for m in range(NCH):
            asm = asm_pool.tile([P, HW], BF16, tag="asm")
            for g in range(NSP // 4):
                # two psum tiles of 1024 cols (2 banks) per group of 4
                # n-tiles; k-outer so one weight load serves 4 matmuls
                ps0 = gpsum.tile([P, 1024], F32, tag="g")
                ps1 = gpsum.tile([P, 1024], F32, tag="g")
                pss = (ps0, ps1)
                for k in range(NCH):
                    for nn in range(4):
                        n = g * 4 + nn
                        nc.tensor.matmul(
                            pss[nn // 2][:, (nn % 2) * 512:(nn % 2 + 1) * 512],
                            wm[:, k, m * P:(m + 1) * P],
                            rhs[k][:, n * 512:(n + 1) * 512],
                            start=(k == 0), stop=(k == NCH - 1))
                for hh in range(2):
                    n0 = g * 4 + hh * 2
                    nc.vector.scalar_tensor_tensor(
                        out=asm[:, n0 * 512:(n0 + 2) * 512],
                        in0=xdir[m][:, n0 * 512:(n0 + 2) * 512],
                        scalar=atts[m][:], in1=pss[hh][:],
                        op0=ALU.mult, op1=ALU.add)
            out_eng.dma_start(odram[b, m * P:(m + 1) * P, :], asm[:])

    # ---- per-batch pipeline ----
    for b in range(B):
        # t branch: loads, exact means, SE, top-k mask
        xt, hs_t = load_chunks(x_t, b, xt_pool, "t")
        sums_t = make_sums(hs_t, "t")
        z_t, att_t = _se_branch(nc, pools, w1t, b1t, w2t, b2t, sums_t, "t")

        z_row = _row_of(nc, pools, z_t, ident, "zt", F32)
        z_bc = med.tile([P, C], F32, tag="zbc")
        nc.gpsimd.partition_broadcast(z_bc[:], z_row[:])
        rank_ps = psmall.tile([1, C], F32, tag="ps")
        for j in range(NCH):
            cmp = med.tile([P, C], FP16, tag="cmp")
            # cmp[p, f] = 1.0 iff z[f] < z[j*128+p]
            nc.vector.tensor_scalar(cmp[:], z_bc[:], z_t[j][:], None, op0=ALU.is_lt)
            nc.tensor.matmul(rank_ps[:], ones_col[:], cmp[:],
                             start=(j == 0), stop=(j == NCH - 1))
        mask_row = small.tile([1, C], BF16, tag="maskrow")
        nc.vector.tensor_scalar(mask_row[:], rank_ps[:], float(K_TOP), None,
                                op0=ALU.is_lt)
        mask_bc = med.tile([P, C], BF16, tag="maskbc")
        nc.gpsimd.partition_broadcast(mask_bc[:], mask_row[:])
        wtm = wm_pool.tile([P, NCH, C], BF16, tag="wtm")
        for j in range(NCH):
            nc.vector.tensor_mul(wtm[:, j, :], wt_full[:, j, :], mask_bc[:])

        # c branch: loads, means, SE, weight scaling
        xc, hs_c = load_chunks(x_c, b, xc_pool, "c")
        sums_c = make_sums(hs_c, "c")
        _, att_c = _se_branch(nc, pools, w1c, b1c, w2c, b2c, sums_c, "c")

        attc_row = _row_of(nc, pools, att_c, ident, "ac", BF16)
        attc_bc = med.tile([P, C], BF16, tag="attcbc")
        nc.gpsimd.partition_broadcast(attc_bc[:], attc_row[:])
        wtc = wm_pool.tile([P, NCH, C], BF16, tag="wtc")
        for j in range(NCH):
            nc.vector.tensor_mul(wtc[:, j, :], wt_full[:, j, :], attc_bc[:])

        # out_t[m,n] = att_t[m]*x_t[m,n] + sum_k (mask*W^T)[k,m] @ x_c[k,n]
        # out_c[m,n] = att_c[m]*x_c[m,n] + sum_k (att_c*W^T)[k,m] @ x_t[k,n]
        jobs = ((wtm, xc, att_t, xt, out_t),
                (wtc, xt, att_c, xc, out_c))
        for wm, rhs, atts, xdir, odram in jobs:
            # batch 1 outputs ride the sync HWDGE queue (idle once inputs
            # finish, and ~1.5us faster completion than SWDGE -- the last
            # output's completion feeds the kernel-tail barrier directly)
            gemm_job(b, wm, rhs, atts, xdir, odram,
                     out_eng=nc.sync if b == 1 else None)


def get_program():
    if "nc" not in _CACHE:
        _CACHE["nc"] = build_program()
    return _CACHE["nc"]


def kernel(x_t, x_c, w1_t, b1_t, w2_t, b2_t, w1_c, b1_c, w2_c, b2_c, cross_att):
    global LAST_RESULTS
    nc = get_program()
    params = dict(w1_t=w1_t, b1_t=b1_t, w2_t=w2_t, b2_t=b2_t,
                  w1_c=w1_c, b1_c=b1_c, w2_c=w2_c, b2_c=b2_c,
                  cross_att=cross_att)
    params = {k: np.ascontiguousarray(np.asarray(v, np.float32))
              for k, v in params.items()}
    x_t = np.ascontiguousarray(np.asarray(x_t, np.float32))
    x_c = np.ascontiguousarray(np.asarray(x_c, np.float32))
    in_maps = []
    for core in range(N_CORES):
        sl = slice(core * B, (core + 1) * B)
        in_maps.append({"x_t": x_t[sl], "x_c": x_c[sl], **params})
    res = run_bass_kernel_spmd(
        nc, in_maps, list(range(N_CORES)),
        trace=bool(os.environ.get("KERNEL_TRACE")),
    )
    LAST_RESULTS = res
    out_t = np.concatenate([r["out_t"] for r in res.results], axis=0).astype(np.float32)
    out_c = np.concatenate([r["out_c"] for r in res.results], axis=0).astype(np.float32)
    return out_t, out_c


# revision 44
# speedup vs baseline: 1.0818x; 1.0818x over previous
"""DualBranchCFCA Trainium2 kernel (v3 — engine-separated pipeline).

Math (per batch b):
    att_t = sigmoid(relu(mean_hw(x_t) @ w1_t + b1_t) @ w2_t + b2_t)      [ct]
    att_c = sigmoid(relu(mean_hw(x_c) @ w1_c + b1_c) @ w2_c + b2_c)      [cc]
    mask  = top_k(att_t, K) one-hot mask in {0,1}                        [ct]
    W     = softmax(cross_att, axis=-1)                                  [ct, cc]
    out_t = att_t * x_t + mask  * (W @ x_c)
    out_c = att_c * x_c + att_c * (W @ x_t)

Strategy: data-parallel over batch across 8 cores (2 batches/core), params
replicated.  Per core, strict engine separation so nothing blocks the input
stream:
  - sync engine: issues ONLY the input half-chunk (1 MB) DMAs -> landing ring.
  - ACT engine:  fp32->bf16 casts (with exact fp32 spatial sums via the
    activation accumulator) + the tiny SE MLP nonlinearities + softmax exps.
  - DVE: softmax scale + wt_full/psum-row evacuations + mask compares + the
    fused drain scalar_tensor_tensor (x*att + psum -> bf16 asm tiles),
    1024 cols (2 PSUM banks) per op.
  - gpsimd: partition broadcasts + output DMA issues (SWDGE queue).
  - PE: SE/rank/transpose smalls + the main bf16 GEMMs (k-outer weight reuse,
    fp32 PSUM accumulation over the 4 channel chunks).
Both batches' bf16 chunks stay resident so batch 1 loads stream during
batch 0 GEMMs.  Selection math (top-k over SE logits) is exact fp32.
"""

import os
from contextlib import ExitStack

import numpy as np

import concourse.bacc as bacc
import concourse.bass as bass
import concourse.mybir as mybir
import concourse.tile as tile
from concourse import masks
from concourse.bass_utils import run_bass_kernel_spmd

F32 = mybir.dt.float32
BF16 = mybir.dt.bfloat16
FP16 = mybir.dt.float16
AF = mybir.ActivationFunctionType
ALU = mybir.AluOpType
AX = mybir.AxisListType

N_CORES = 8
B_FULL = 16
B = B_FULL // N_CORES  # batches per core
C = 512                # channels (both branches)
HW = 64 * 64           # flattened spatial
RED = 16
H = C // RED           # SE hidden dim = 32
K_TOP = int(C * 0.3)   # 153
P = 128                # partitions
NCH = C // P           # 4 channel chunks
NHALF = 2              # landing halves per chunk
HALF = HW // NHALF     # 2048
NSP = HW // 512        # 8 spatial tiles of 512

_CACHE = {}
LAST_RESULTS = None  # BassKernelResults of the most recent run (for profiling)


def _se_branch(nc, pools, w1, b1, w2, b2, sums, pfx):
    """SE MLP from per-chunk spatial sums -> (z chunks, att chunks), each [128,1].

    z = sums/HW @ w1 -> relu(+b1) -> @ w2 + b2 (pre-sigmoid logits, exact fp32)
    att = sigmoid(z)
    """
    small, psmall = pools["small"], pools["psmall"]
    hz = psmall.tile([H, 1], F32, tag="ps")
    for i in range(NCH):
        nc.tensor.matmul(hz[:], w1[:, i, :], sums[i][:],
                         start=(i == 0), stop=(i == NCH - 1))
    h_sb = small.tile([H, 1], F32, tag=f"{pfx}h")
    nc.scalar.activation(h_sb[:], hz[:], AF.Relu, bias=b1[:], scale=1.0 / HW)

    z_chunks, att_chunks = [], []
    for j in range(NCH):
        zp = psmall.tile([P, 1], F32, tag="ps")
        nc.tensor.matmul(zp[:], w2[:, j * P:(j + 1) * P], h_sb[:])
        z = small.tile([P, 1], F32, tag=f"{pfx}z{j}")
        nc.scalar.activation(z[:], zp[:], AF.Identity, bias=b2[:, j:j + 1])
        att = small.tile([P, 1], F32, tag=f"{pfx}a{j}")
        nc.scalar.activation(att[:], z[:], AF.Sigmoid)
        z_chunks.append(z)
        att_chunks.append(att)
    return z_chunks, att_chunks


def _row_of(nc, pools, cols, ident, pfx, dtype):
    """Transpose NCH [128,1] column tiles into one [1, C] row tile."""
    small, psmall = pools["small"], pools["psmall"]
    row = small.tile([1, C], dtype, tag=f"{pfx}row")
    for j in range(NCH):
        tp = psmall.tile([1, P], F32, tag="ps")
        nc.tensor.transpose(tp[:], cols[j][:], ident[:])
        nc.vector.tensor_copy(row[:, j * P:(j + 1) * P], tp[:])
    return row


def build_program():
    nc = bacc.Bacc("TRN2", target_bir_lowering=False, debug=False)

    def din(name, shape):
        return nc.dram_tensor(name, shape, F32, kind="ExternalInput").ap()

    x_t = din("x_t", [B, C, 64, 64]).rearrange("b c h w -> b c (h w)")
    x_c = din("x_c", [B, C, 64, 64]).rearrange("b c h w -> b c (h w)")
    w1_t, b1_t = din("w1_t", [C, H]), din("b1_t", [H])
    w2_t, b2_t = din("w2_t", [H, C]), din("b2_t", [C])
    w1_c, b1_c = din("w1_c", [C, H]), din("b1_c", [H])
    w2_c, b2_c = din("w2_c", [H, C]), din("b2_c", [C])
    cross_att = din("cross_att", [C, C])

    out_t = nc.dram_tensor("out_t", [B, C, 64, 64], BF16,
                           kind="ExternalOutput").ap().rearrange("b c h w -> b c (h w)")
    out_c = nc.dram_tensor("out_c", [B, C, 64, 64], BF16,
                           kind="ExternalOutput").ap().rearrange("b c h w -> b c (h w)")

    with tile.TileContext(nc) as tc:
        with ExitStack() as ctx:
            _body(ctx, tc, x_t, x_c, w1_t, b1_t, w2_t, b2_t,
                  w1_c, b1_c, w2_c, b2_c, cross_att, out_t, out_c)
    nc.compile()
    return nc


def _body(ctx, tc, x_t, x_c, w1_t, b1_t, w2_t, b2_t,
          w1_c, b1_c, w2_c, b2_c, cross_att, out_t, out_c):
    nc = tc.nc
    const = ctx.enter_context(tc.tile_pool(name="const", bufs=1))
    small = ctx.enter_context(tc.tile_pool(name="small", bufs=2))
    med = ctx.enter_context(tc.tile_pool(name="med", bufs=2))
    wm_pool = ctx.enter_context(tc.tile_pool(name="wm", bufs=1))
    land_pool = ctx.enter_context(tc.tile_pool(name="land", bufs=3))
    xt_pool = ctx.enter_context(tc.tile_pool(name="xt", bufs=2 * NCH))
    xc_pool = ctx.enter_context(tc.tile_pool(name="xc", bufs=2 * NCH))
    asm_pool = ctx.enter_context(tc.tile_pool(name="asm", bufs=2))
    psmall = ctx.enter_context(tc.tile_pool(name="psmall", bufs=2, space="PSUM"))
    gpsum = ctx.enter_context(tc.tile_pool(name="gpsum", bufs=3, space="PSUM"))
    pools = {"small": small, "psmall": psmall}

    # ---- constants ----
    ident = const.tile([P, P], F32)
    masks.make_identity(nc, ident[:])
    ones_col = const.tile([P, 1], FP16)
    nc.vector.memset(ones_col[:], 1.0)
    # dummy broadcast at t=0: preloads the gpsimd Q7 partition_broadcast
    # program during the idle head so batch 0's mask-chain broadcast doesn't
    # pay the ~9us first-use program-load stall on the critical path
    warm_bc = const.tile([P, P], F32, tag="warmbc")
    nc.gpsimd.partition_broadcast(warm_bc[:], ident[0:1, :])

    # SE weights: w1 as [128, NCH, H] (lhsT chunks over contraction dim c),
    # w2 as [H, C] (lhsT over contraction dim h), biases as columns.
    def load_se(w1d, b1d, w2d, b2d, pfx):
        w1 = const.tile([P, NCH, H], F32, tag=f"{pfx}w1")
        nc.sync.dma_start(w1[:], w1d.rearrange("(k p) h -> p k h", p=P))
        b1 = const.tile([H, 1], F32, tag=f"{pfx}b1")
        nc.sync.dma_start(b1[:], b1d.unsqueeze(1))
        w2 = const.tile([H, C], F32, tag=f"{pfx}w2")
        nc.sync.dma_start(w2[:], w2d)
        b2 = const.tile([P, NCH], F32, tag=f"{pfx}b2")
        nc.sync.dma_start(b2[:], b2d.rearrange("(k p) -> p k", p=P))
        return w1, b1, w2, b2

    w1t, b1t, w2t, b2t = load_se(w1_t, b1_t, w2_t, b2_t, "t")
    w1c, b1c, w2c, b2c = load_se(w1_c, b1_c, w2_c, b2_c, "c")

    # ---- softmax(cross_att), transpose -> wt_full[c_part, j, t] (bf16) ----
    # No max-subtraction needed: |cross_att| <~ 5 so exp stays in fp32 range
    # and softmax is shift-invariant.  Transposes ride the big psum ring and
    # evacuate on DVE so the ACT engine stays free for the input casts.
    wt_full = const.tile([P, NCH, C], BF16, tag="wt")
    for i in range(NCH):
        ca = med.tile([P, C], F32, tag="zbc")
        nc.sync.dma_start(ca[:], cross_att[i * P:(i + 1) * P, :])
        sumexp = small.tile([P, 1], F32, tag="sumexp")
        nc.scalar.activation(ca[:], ca[:], AF.Exp, accum_out=sumexp[:])
        rec = small.tile([P, 1], F32, tag="rec")
        nc.vector.reciprocal(rec[:], sumexp[:])
        nc.vector.tensor_scalar_mul(ca[:], ca[:], rec[:])
        for j in range(NCH):
            tp = gpsum.tile([P, P], F32, tag="g")
            nc.tensor.transpose(tp[:], ca[:, j * P:(j + 1) * P], ident[:])
            nc.vector.tensor_copy(wt_full[:, j, i * P:(i + 1) * P], tp[:])

    def load_chunks(xdram, b, pool, pfx):
        """Stream half-chunks (1 MB) through the landing ring on the sync
        HWDGE queue; one fused ACT op per half casts to bf16 and emits the
        exact fp32 half-sum (ACT accumulates pre-cast)."""
        bf_chunks, half_sums = [], []
        for i in range(NCH):
            xb = pool.tile([P, HW], BF16, tag=f"{pfx}bf")
            halves = []
            for hh in range(NHALF):
                land = land_pool.tile([P, HALF], F32, tag="land")
                nc.sync.dma_start(
                    land[:], xdram[b, i * P:(i + 1) * P,
                                   hh * HALF:(hh + 1) * HALF])
                sh = small.tile([P, 1], F32, tag=f"{pfx}sh{i}{hh}")
                nc.scalar.activation(xb[:, hh * HALF:(hh + 1) * HALF], land[:],
                                     AF.Copy, accum_out=sh[:])
                halves.append(sh)
            bf_chunks.append(xb)
            half_sums.append(halves)
        return bf_chunks, half_sums

    def make_sums(half_sums, pfx):
        sums = []
        for i, halves in enumerate(half_sums):
            s = small.tile([P, 1], F32, tag=f"{pfx}s{i}")
            nc.vector.tensor_add(s[:], halves[0][:], halves[1][:])
            sums.append(s)
        return sums

    def gemm_job(b, wm, rhs, atts, xdir, odram):
        # out[m,n] = atts[m]*xdir[m,n] + sum_k wm[k,m] @ rhs[k,n]
        for m in range(NCH):
            asm = asm_pool.tile([P, HW], BF16, tag="asm")
            for g in range(NSP // 4):
                # two psum tiles of 1024 cols (2 banks) per group of 4
                # n-tiles; k-outer so one weight load serves 4 matmuls
                ps0 = gpsum.tile([P, 1024], F32, tag="g")
                ps1 = gpsum.tile([P, 1024], F32, tag="g")
                pss = (ps0, ps1)
                for k in range(NCH):
                    for nn in range(4):
                        n = g * 4 + nn
                        nc.tensor.matmul(
                            pss[nn // 2][:, (nn % 2) * 512:(nn % 2 + 1) * 512],
                            wm[:, k, m * P:(m + 1) * P],
                            rhs[k][:, n * 512:(n + 1) * 512],
                            start=(k == 0), stop=(k == NCH - 1))
                for hh in range(2):
                    n0 = g * 4 + hh * 2
                    nc.vector.scalar_tensor_tensor(
                        out=asm[:, n0 * 512:(n0 + 2) * 512],
                        in0=xdir[m][:, n0 * 512:(n0 + 2) * 512],
                        scalar=atts[m][:], in1=pss[hh][:],
                        op0=ALU.mult, op1=ALU.add)
            nc.gpsimd.dma_start(odram[b, m * P:(m + 1) * P, :], asm[:])

    # ---- per-batch pipeline ----
    for b in range(B):
        # t branch: loads, exact means, SE, top-k mask
        xt, hs_t = load_chunks(x_t, b, xt_pool, "t")
        sums_t = make_sums(hs_t, "t")
        z_t, att_t = _se_branch(nc, pools, w1t, b1t, w2t, b2t, sums_t, "t")

        z_row = _row_of(nc, pools, z_t, ident, "zt", F32)
        z_bc = med.tile([P, C], F32, tag="zbc")
        nc.gpsimd.partition_broadcast(z_bc[:], z_row[:])
        rank_ps = psmall.tile([1, C], F32, tag="ps")
        for j in range(NCH):
            cmp = med.tile([P, C], FP16, tag="cmp")
            # cmp[p, f] = 1.0 iff z[f] < z[j*128+p]
            nc.vector.tensor_scalar(cmp[:], z_bc[:], z_t[j][:], None, op0=ALU.is_lt)
            nc.tensor.matmul(rank_ps[:], ones_col[:], cmp[:],
                             start=(j == 0), stop=(j == NCH - 1))
        mask_row = small.tile([1, C], BF16, tag="maskrow")
        nc.vector.tensor_scalar(mask_row[:], rank_ps[:], float(K_TOP), None,
                                op0=ALU.is_lt)
        mask_bc = med.tile([P, C], BF16, tag="maskbc")
        nc.gpsimd.partition_broadcast(mask_bc[:], mask_row[:])
        wtm = wm_pool.tile([P, NCH, C], BF16, tag="wtm")
        for j in range(NCH):
            nc.vector.tensor_mul(wtm[:, j, :], wt_full[:, j, :], mask_bc[:])

        # c branch: loads, means, SE, weight scaling
        xc, hs_c = load_chunks(x_c, b, xc_pool, "c")
        sums_c = make_sums(hs_c, "c")
        _, att_c = _se_branch(nc, pools, w1c, b1c, w2c, b2c, sums_c, "c")

        attc_row = _row_of(nc, pools, att_c, ident, "ac", BF16)
        attc_bc = med.tile([P, C], BF16, tag="attcbc")
        nc.gpsimd.partition_broadcast(attc_bc[:], attc_row[:])
        wtc = wm_pool.tile([P, NCH, C], BF16, tag="wtc")
        for j in range(NCH):
            nc.vector.tensor_mul(wtc[:, j, :], wt_full[:, j, :], attc_bc[:])

        # out_t[m,n] = att_t[m]*x_t[m,n] + sum_k (mask*W^T)[k,m] @ x_c[k,n]
        # out_c[m,n] = att_c[m]*x_c[m,n] + sum_k (att_c*W^T)[k,m] @ x_t[k,n]
        jobs = ((wtm, xc, att_t, xt, out_t),
                (wtc, xt, att_c, xc, out_c))
        for wm, rhs, atts, xdir, odram in jobs:
            gemm_job(b, wm, rhs, atts, xdir, odram)


def get_program():
    if "nc" not in _CACHE:
        _CACHE["nc"] = build_program()
    return _CACHE["nc"]


def kernel(x_t, x_c, w1_t, b1_t, w2_t, b2_t, w1_c, b1_c, w2_c, b2_c, cross_att):
    global LAST_RESULTS
    nc = get_program()
    params = dict(w1_t=w1_t, b1_t=b1_t, w2_t=w2_t, b2_t=b2_t,
                  w1_c=w1_c, b1_c=b1_c, w2_c=w2_c, b2_c=b2_c,
                  cross_att=cross_att)
    params = {k: np.ascontiguousarray(np.asarray(v, np.float32))
              for k, v in params.items()}
    x_t = np.ascontiguousarray(np.asarray(x_t, np.float32))
    x_c = np.ascontiguousarray(np.asarray(x_c, np.float32))
    in_maps = []
    for core in range(N_CORES):
        sl = slice(core * B, (core + 1) * B)
        in_maps.append({"x_t": x_t[sl], "x_c": x_c[sl], **params})
    res = run_bass_kernel_spmd(
        nc, in_maps, list(range(N_CORES)),
        trace=bool(os.environ.get("KERNEL_TRACE")),
    )
    LAST_RESULTS = res
    out_t = np.concatenate([r["out_t"] for r in res.results], axis=0).astype(np.float32)
    out_c = np.concatenate([r["out_c"] for r in res.results], axis=0).astype(np.float32)
    return out_t, out_c
